# revision 31
# baseline (speedup 1.0000x reference)
"""Trainium2 Bass kernel for masked 15-bin Expected Calibration Error.

Contract: kernel(**full_inputs) -> full output (scalar f32), inputs are the
four full [8192, 4096] tensors. Internally: row-shard across 8 NeuronCores
(data-parallel, 1024 rows each); each core computes per-partition partial
cumulative bin sums L_t = sum((bin > t) * w * (conf - correct)) for
t=0..14; the host reduces the tiny partials, adds sum(mask) (a cheap host
reduction), and finishes:

    ece = sum_b |L_b - L_{b+1}| / sum(w)

which equals the reference sum_b |avg_conf_b - acc_b| * n_b / total since
the n_b/safe_b factors cancel for non-empty bins and empty bins contribute
exactly zero to both.

Device program per [128 x 2048] tile:
  ACT:  u  = bf16(15*conf + 127.5)   exact integer bin code 127 + ceil(15c)
        (bf16 ulp is 1.0 on [128,256), so the f32->bf16 round-to-nearest
        lands exactly on the bin integer; boundary ties are measure-zero
        for random f32 input)
  DVE:  corr = (pred == targ); uw = (mask > 0.5) * u; d = conf - corr
  DVE+GPSIMD: 15x fused scalar_tensor_tensor threshold passes
        out = (uw > 127+t) * d, accum_out -> per-partition L_t column;
        7 thresholds run on GPSIMD, 8 on DVE (balanced per the cost model,
        ~1.9x faster than DVE alone).
Masked-out elements (w=0) have uw=0 and c==0 gives uw=127, both below
every threshold, so no separate in-range masking is needed.

pred/targets are packed host-side into one [ROWS, 2, COLS] int32 tensor
(fewer, larger DMAs).
"""

import os
import sys

for _p in ("/opt/trn_rl_repo",):
    if _p not in sys.path and os.path.isdir(_p):
        sys.path.insert(0, _p)

import numpy as np

import concourse.bacc as bacc
import concourse.mybir as mybir
import concourse.tile as tile
from concourse.bass_utils import run_bass_kernel_spmd

N_CORES = 8
N_BINS = 15
FULL_ROWS = 8192
COLS = 4096
ROWS = FULL_ROWS // N_CORES   # 1024 rows per core
FREE = 2048                   # free-dim tile size
P = 128                       # SBUF partitions
N_GP = 0                      # GPSIMD cannot run scalar_tensor_tensor (ISA)

LAST_EXEC_TIME_NS = None
LAST_RESULTS = None
_CACHE = {}


def _build_program(rows=ROWS, cols=COLS, free=FREE, num_devices=N_CORES,
                   n_gp=N_GP):
    n_r = rows // P
    n_c = cols // free
    n_tiles = n_r * n_c

    nc = bacc.Bacc(
        "TRN2", target_bir_lowering=False, debug=False, num_devices=num_devices
    )

    f32 = mybir.dt.float32
    bf16 = mybir.dt.bfloat16
    i32 = mybir.dt.int32

    conf = nc.dram_tensor("confidences", [rows, cols], f32, kind="ExternalInput").ap()
    pt = nc.dram_tensor("pt", [rows, 2, cols], i32, kind="ExternalInput").ap()
    wm = nc.dram_tensor("wm", [rows, cols], i32, kind="ExternalInput").ap()
    outL = nc.dram_tensor(
        "partL", [P, n_tiles * N_BINS], f32, kind="ExternalOutput"
    ).ap()
    outG = nc.dram_tensor(
        "partG", [P, n_tiles * N_BINS], f32, kind="ExternalOutput"
    ).ap()

    Alu = mybir.AluOpType
    Act = mybir.ActivationFunctionType

    with tile.TileContext(nc) as tc:
        with (
            tc.tile_pool(name="in_f", bufs=3) as in_f,
            tc.tile_pool(name="in_i", bufs=3) as in_i,
            tc.tile_pool(name="work", bufs=2) as work,
            tc.tile_pool(name="stage", bufs=1) as stage_pool,
        ):
            # Persistent per-tile accumulator columns, one tensor per
            # writing engine.
            Ldve = stage_pool.tile([P, n_tiles * N_BINS], f32, tag="Ldve")
            Lgp = stage_pool.tile([P, n_tiles * N_BINS], f32, tag="Lgp")
            # Each engine writes only its share of every 15 columns; zero
            # both tensors so the full range DMAs out defined.
            nc.vector.memset(Ldve[:], 0.0)
            nc.vector.memset(Lgp[:], 0.0)

            for it in range(n_tiles):
                r0 = (it // n_c) * P
                c0 = (it % n_c) * free

                c_t = in_f.tile([P, free], f32, tag="c")
                i_t = in_i.tile([P, 2, free], i32, tag="pt")
                w_t = in_i.tile([P, free], i32, tag="wm")
                nc.sync.dma_start(c_t[:], conf[r0 : r0 + P, c0 : c0 + free])
                nc.sync.dma_start(i_t[:], pt[r0 : r0 + P, :, c0 : c0 + free])
                nc.sync.dma_start(w_t[:], wm[r0 : r0 + P, c0 : c0 + free])

                u_t = work.tile([P, free], bf16, tag="u")
                corr_t = work.tile([P, free], bf16, tag="corr")
                uw_t = work.tile([P, free], bf16, tag="uw")
                d_t = work.tile([P, free], bf16, tag="d")
                scr_t = work.tile([P, free], bf16, tag="scr")
                gscr_t = work.tile([P, free], bf16, tag="gscr")

                nc.scalar.activation(
                    u_t[:], c_t[:], Act.Copy, bias=127.5, scale=15.0
                )
                nc.vector.tensor_tensor(corr_t[:], i_t[:, 0], i_t[:, 1], Alu.is_equal)
                nc.vector.scalar_tensor_tensor(
                    uw_t[:], w_t[:], 0.5, u_t[:], Alu.is_gt, Alu.mult
                )
                nc.vector.tensor_tensor(d_t[:], c_t[:], corr_t[:], Alu.subtract)

                for t in range(N_BINS):
                    col = slice(it * N_BINS + t, it * N_BINS + t + 1)
                    if t < n_gp:
                        nc.gpsimd.scalar_tensor_tensor(
                            gscr_t[:], uw_t[:], float(127 + t), d_t[:],
                            Alu.is_gt, Alu.mult, accum_out=Lgp[:, col],
                        )
                    else:
                        nc.vector.scalar_tensor_tensor(
                            scr_t[:], uw_t[:], float(127 + t), d_t[:],
                            Alu.is_gt, Alu.mult, accum_out=Ldve[:, col],
                        )

            nc.sync.dma_start(outL[:, :], Ldve[:])
            nc.sync.dma_start(outG[:, :], Lgp[:])

    # Bacc lowering: splits multi-wait sync conditions into EventSemaphore
    # instructions (the HW encodes one wait per instruction) and the rest
    # of the pre-walrus pipeline.
    nc.compile()
    return nc, n_tiles


def _get_program():
    if "prog" not in _CACHE:
        _CACHE["prog"] = _build_program()
    return _CACHE["prog"]


def _combine(partL_list, partG_list, total):
    L = np.zeros(N_BINS, dtype=np.float64)
    for pl, pg in zip(partL_list, partG_list):
        s = np.asarray(pl).astype(np.float64) + np.asarray(pg).astype(np.float64)
        L += s.reshape(s.shape[0], -1, N_BINS).sum(axis=(0, 1))
    delta = L.copy()
    delta[:-1] -= L[1:]
    return np.float32(np.abs(delta).sum() / total)


def kernel(confidences, predictions, targets, mask):
    global LAST_EXEC_TIME_NS, LAST_RESULTS
    nc, n_tiles = _get_program()

    conf = np.ascontiguousarray(np.asarray(confidences, dtype=np.float32))
    pred = np.asarray(predictions, dtype=np.int32)
    targ = np.asarray(targets, dtype=np.int32)
    msk = np.ascontiguousarray(np.asarray(mask, dtype=np.int32))
    assert conf.shape == (FULL_ROWS, COLS)

    pt = np.ascontiguousarray(np.stack([pred, targ], axis=1))

    in_maps = []
    for i in range(N_CORES):
        sl = slice(i * ROWS, (i + 1) * ROWS)
        in_maps.append({"confidences": conf[sl], "pt": pt[sl], "wm": msk[sl]})

    trace = bool(int(os.environ.get("ECE_TRACE", "0")))
    res = run_bass_kernel_spmd(nc, in_maps, list(range(N_CORES)), trace=trace)
    LAST_EXEC_TIME_NS = res.exec_time_ns
    LAST_RESULTS = res

    total = float(msk.sum(dtype=np.int64))
    return _combine(
        [res.results[i]["partL"] for i in range(N_CORES)],
        [res.results[i]["partG"] for i in range(N_CORES)],
        total,
    )


# revision 32
# speedup vs baseline: 1.0001x; 1.0001x over previous
"""Trainium2 Bass kernel for masked 15-bin Expected Calibration Error.

Contract: kernel(**full_inputs) -> full output (scalar f32), inputs are the
four full [8192, 4096] tensors. Internally: row-shard across 8 NeuronCores
(data-parallel, 1024 rows each); each core computes per-partition partial
cumulative bin sums L_t = sum((bin > t) * w * (conf - correct)) for
t=0..14; the host reduces the tiny partials, adds sum(mask) (a cheap host
reduction), and finishes:

    ece = sum_b |L_b - L_{b+1}| / sum(w)

which equals the reference sum_b |avg_conf_b - acc_b| * n_b / total since
the n_b/safe_b factors cancel for non-empty bins and empty bins contribute
exactly zero to both.

Device program per [128 x 2048] tile:
  ACT:  u  = bf16(15*conf + 127.5)   exact integer bin code 127 + ceil(15c)
        (bf16 ulp is 1.0 on [128,256), so the f32->bf16 round-to-nearest
        lands exactly on the bin integer; boundary ties are measure-zero
        for random f32 input)
  DVE:  corr = (pred == targ); uw = (mask > 0.5) * u; d = conf - corr
  DVE:  15x fused scalar_tensor_tensor threshold passes
        out = (uw > 127+t) * d, accum_out -> per-partition L_t column.
        (GPSIMD/Pool cannot execute TT/STT on this toolchain - walrus
        engine check - so all element work stays on DVE.)
Masked-out elements (w=0) have uw=0 and c==0 gives uw=127, both below
every threshold, so no separate in-range masking is needed.

pred/targets are packed host-side into one [ROWS, 2, COLS] int32 tensor
(fewer, larger DMAs).
"""

import os
import sys

for _p in ("/opt/trn_rl_repo",):
    if _p not in sys.path and os.path.isdir(_p):
        sys.path.insert(0, _p)

import numpy as np

import concourse.bacc as bacc
import concourse.mybir as mybir
import concourse.tile as tile
from concourse.bass_utils import run_bass_kernel_spmd

N_CORES = 8
N_BINS = 15
FULL_ROWS = 8192
COLS = 4096
ROWS = FULL_ROWS // N_CORES   # 1024 rows per core
FREE = 2048                   # free-dim tile size
P = 128                       # SBUF partitions
LAST_EXEC_TIME_NS = None
LAST_RESULTS = None
_CACHE = {}


def _build_program(rows=ROWS, cols=COLS, free=FREE, num_devices=N_CORES):
    n_r = rows // P
    n_c = cols // free
    n_tiles = n_r * n_c

    nc = bacc.Bacc(
        "TRN2", target_bir_lowering=False, debug=False, num_devices=num_devices
    )

    f32 = mybir.dt.float32
    bf16 = mybir.dt.bfloat16
    i32 = mybir.dt.int32

    conf = nc.dram_tensor("confidences", [rows, cols], f32, kind="ExternalInput").ap()
    pt = nc.dram_tensor("pt", [rows, 2, cols], i32, kind="ExternalInput").ap()
    wm = nc.dram_tensor("wm", [rows, cols], i32, kind="ExternalInput").ap()
    outL = nc.dram_tensor(
        "partL", [P, n_tiles * N_BINS], f32, kind="ExternalOutput"
    ).ap()

    Alu = mybir.AluOpType
    Act = mybir.ActivationFunctionType

    with tile.TileContext(nc) as tc:
        with (
            tc.tile_pool(name="in_f", bufs=3) as in_f,
            tc.tile_pool(name="in_i", bufs=3) as in_i,
            tc.tile_pool(name="work", bufs=2) as work,
            tc.tile_pool(name="stage", bufs=1) as stage_pool,
        ):
            # Persistent per-tile accumulator columns (DVE-written; every
            # column is written exactly once, so no zeroing needed).
            Ldve = stage_pool.tile([P, n_tiles * N_BINS], f32, tag="Ldve")

            for it in range(n_tiles):
                r0 = (it // n_c) * P
                c0 = (it % n_c) * free

                c_t = in_f.tile([P, free], f32, tag="c")
                i_t = in_i.tile([P, 2, free], i32, tag="pt")
                w_t = in_i.tile([P, free], i32, tag="wm")
                nc.sync.dma_start(c_t[:], conf[r0 : r0 + P, c0 : c0 + free])
                nc.sync.dma_start(i_t[:], pt[r0 : r0 + P, :, c0 : c0 + free])
                nc.sync.dma_start(w_t[:], wm[r0 : r0 + P, c0 : c0 + free])

                u_t = work.tile([P, free], bf16, tag="u")
                corr_t = work.tile([P, free], bf16, tag="corr")
                uw_t = work.tile([P, free], bf16, tag="uw")
                d_t = work.tile([P, free], bf16, tag="d")
                scr_t = work.tile([P, free], bf16, tag="scr")

                nc.scalar.activation(
                    u_t[:], c_t[:], Act.Copy, bias=127.5, scale=15.0
                )
                nc.vector.tensor_tensor(corr_t[:], i_t[:, 0], i_t[:, 1], Alu.is_equal)
                nc.vector.scalar_tensor_tensor(
                    uw_t[:], w_t[:], 0.5, u_t[:], Alu.is_gt, Alu.mult
                )
                nc.vector.tensor_tensor(d_t[:], c_t[:], corr_t[:], Alu.subtract)

                for t in range(N_BINS):
                    col = slice(it * N_BINS + t, it * N_BINS + t + 1)
                    nc.vector.scalar_tensor_tensor(
                        scr_t[:], uw_t[:], float(127 + t), d_t[:],
                        Alu.is_gt, Alu.mult, accum_out=Ldve[:, col],
                    )

            nc.sync.dma_start(outL[:, :], Ldve[:])

    # Bacc lowering: splits multi-wait sync conditions into EventSemaphore
    # instructions (the HW encodes one wait per instruction) and the rest
    # of the pre-walrus pipeline.
    nc.compile()
    return nc, n_tiles


def _get_program():
    if "prog" not in _CACHE:
        _CACHE["prog"] = _build_program()
    return _CACHE["prog"]


def _combine(partL_list, total):
    L = np.zeros(N_BINS, dtype=np.float64)
    for pl in partL_list:
        s = np.asarray(pl).astype(np.float64)
        L += s.reshape(s.shape[0], -1, N_BINS).sum(axis=(0, 1))
    delta = L.copy()
    delta[:-1] -= L[1:]
    return np.float32(np.abs(delta).sum() / total)


def kernel(confidences, predictions, targets, mask):
    global LAST_EXEC_TIME_NS, LAST_RESULTS
    nc, n_tiles = _get_program()

    conf = np.ascontiguousarray(np.asarray(confidences, dtype=np.float32))
    pred = np.asarray(predictions, dtype=np.int32)
    targ = np.asarray(targets, dtype=np.int32)
    msk = np.ascontiguousarray(np.asarray(mask, dtype=np.int32))
    assert conf.shape == (FULL_ROWS, COLS)

    pt = np.ascontiguousarray(np.stack([pred, targ], axis=1))

    in_maps = []
    for i in range(N_CORES):
        sl = slice(i * ROWS, (i + 1) * ROWS)
        in_maps.append({"confidences": conf[sl], "pt": pt[sl], "wm": msk[sl]})

    trace = bool(int(os.environ.get("ECE_TRACE", "0")))
    res = run_bass_kernel_spmd(nc, in_maps, list(range(N_CORES)), trace=trace)
    LAST_EXEC_TIME_NS = res.exec_time_ns
    LAST_RESULTS = res

    total = float(msk.sum(dtype=np.int64))
    return _combine([res.results[i]["partL"] for i in range(N_CORES)], total)


# revision 34
# speedup vs baseline: 1.3723x; 1.3722x over previous
"""Trainium2 Bass kernel for masked 15-bin Expected Calibration Error.

Contract: kernel(**full_inputs) -> full output (scalar f32), inputs are the
four full [8192, 4096] tensors. Internally: row-shard across 8 NeuronCores
(data-parallel, 1024 rows each); each core computes per-partition partial
cumulative bin sums L_t = sum((bin > t) * w * (conf - correct)) for
t=0..14; the host reduces the tiny partials, adds sum(mask) (a cheap host
reduction), and finishes:

    ece = sum_b |L_b - L_{b+1}| / sum(w)

which equals the reference sum_b |avg_conf_b - acc_b| * n_b / total since
the n_b/safe_b factors cancel for non-empty bins and empty bins contribute
exactly zero to both.

Device program per [128 x 2048] tile:
  ACT:  u  = bf16(15*conf + 127.5)   exact integer bin code 127 + ceil(15c)
        (bf16 ulp is 1.0 on [128,256), so the f32->bf16 round-to-nearest
        lands exactly on the bin integer; boundary ties are measure-zero
        for random f32 input)
  DVE:  corr = (pred == targ); uw = (mask > 0.5) * u; d = conf - corr;
        z = 4*uw + d (f32); 9x fused scalar_tensor_tensor threshold passes
        out = (uw > 127+t) * d, accum_out -> per-partition L_t column.
  ACT:  6 thresholds via accumulating relu moments, 2 passes each:
        A_t = sum relu(z - 4*(127.5+t)) and B_t = sum relu(uw - (127.5+t))
        satisfy L_t = A_t - 4*B_t exactly (bin codes are integers), so the
        otherwise-idle scalar engine absorbs 40%% of the threshold work.
        (GPSIMD/Pool cannot execute TT/STT on this toolchain - walrus
        engine check - so it stays idle.)
Masked-out elements (w=0) have uw=0 and c==0 gives uw=127, both below
every threshold, so no separate in-range masking is needed.

pred/targets are packed host-side into one [ROWS, 2, COLS] int32 tensor
(fewer, larger DMAs).
"""

import os
import sys

for _p in ("/opt/trn_rl_repo",):
    if _p not in sys.path and os.path.isdir(_p):
        sys.path.insert(0, _p)

import numpy as np

import concourse.bacc as bacc
import concourse.mybir as mybir
import concourse.tile as tile
from concourse.bass_utils import run_bass_kernel_spmd

N_CORES = 8
N_BINS = 15
FULL_ROWS = 8192
COLS = 4096
ROWS = FULL_ROWS // N_CORES   # 1024 rows per core
FREE = 2048                   # free-dim tile size
P = 128                       # SBUF partitions
N_ACT = 6                     # thresholds computed on ACT via relu moments
KSC = 4.0                     # z = KSC*uw + d encoding scale
LAST_EXEC_TIME_NS = None
LAST_RESULTS = None
_CACHE = {}


def _build_program(rows=ROWS, cols=COLS, free=FREE, num_devices=N_CORES):
    n_r = rows // P
    n_c = cols // free
    n_tiles = n_r * n_c

    nc = bacc.Bacc(
        "TRN2", target_bir_lowering=False, debug=False, num_devices=num_devices
    )

    f32 = mybir.dt.float32
    bf16 = mybir.dt.bfloat16
    i32 = mybir.dt.int32

    conf = nc.dram_tensor("confidences", [rows, cols], f32, kind="ExternalInput").ap()
    pt = nc.dram_tensor("pt", [rows, 2, cols], i32, kind="ExternalInput").ap()
    wm = nc.dram_tensor("wm", [rows, cols], i32, kind="ExternalInput").ap()
    n_dve = N_BINS - N_ACT
    outL = nc.dram_tensor(
        "partL", [P, n_tiles * n_dve], f32, kind="ExternalOutput"
    ).ap()
    outA = nc.dram_tensor(
        "partA", [P, n_tiles * N_ACT], f32, kind="ExternalOutput"
    ).ap()
    outB = nc.dram_tensor(
        "partB", [P, n_tiles * N_ACT], f32, kind="ExternalOutput"
    ).ap()

    Alu = mybir.AluOpType
    Act = mybir.ActivationFunctionType

    with tile.TileContext(nc) as tc:
        with (
            tc.tile_pool(name="in_f", bufs=3) as in_f,
            tc.tile_pool(name="in_i", bufs=3) as in_i,
            tc.tile_pool(name="work", bufs=2) as work,
            tc.tile_pool(name="stage", bufs=1) as stage_pool,
        ):
            # Persistent per-tile accumulator columns; every column is
            # written exactly once. Ldve: DVE threshold sums; LA/LB: the
            # ACT relu-moment families (L_t = A_t - KSC*B_t on the host).
            Ldve = stage_pool.tile([P, n_tiles * n_dve], f32, tag="Ldve")
            LA = stage_pool.tile([P, n_tiles * N_ACT], f32, tag="LA")
            LB = stage_pool.tile([P, n_tiles * N_ACT], f32, tag="LB")
            biasA, biasB = {}, {}
            for j in range(N_ACT):
                t = n_dve + j
                ba = stage_pool.tile([P, 1], f32, tag=f"ba{j}")
                bb = stage_pool.tile([P, 1], f32, tag=f"bb{j}")
                nc.vector.memset(ba[:], -KSC * (127.5 + t))
                nc.vector.memset(bb[:], -(127.5 + t))
                biasA[t], biasB[t] = ba, bb

            for it in range(n_tiles):
                r0 = (it // n_c) * P
                c0 = (it % n_c) * free

                c_t = in_f.tile([P, free], f32, tag="c")
                i_t = in_i.tile([P, 2, free], i32, tag="pt")
                w_t = in_i.tile([P, free], i32, tag="wm")
                nc.sync.dma_start(c_t[:], conf[r0 : r0 + P, c0 : c0 + free])
                nc.sync.dma_start(i_t[:], pt[r0 : r0 + P, :, c0 : c0 + free])
                nc.sync.dma_start(w_t[:], wm[r0 : r0 + P, c0 : c0 + free])

                u_t = work.tile([P, free], bf16, tag="u")
                corr_t = work.tile([P, free], bf16, tag="corr")
                uw_t = work.tile([P, free], bf16, tag="uw")
                d_t = work.tile([P, free], bf16, tag="d")
                z_t = work.tile([P, free], f32, tag="z")
                scr_t = work.tile([P, free], bf16, tag="scr")
                ascr_t = work.tile([P, free], f32, tag="ascr")

                nc.scalar.activation(
                    u_t[:], c_t[:], Act.Copy, bias=127.5, scale=15.0
                )
                nc.vector.tensor_tensor(corr_t[:], i_t[:, 0], i_t[:, 1], Alu.is_equal)
                nc.vector.scalar_tensor_tensor(
                    uw_t[:], w_t[:], 0.5, u_t[:], Alu.is_gt, Alu.mult
                )
                nc.vector.tensor_tensor(d_t[:], c_t[:], corr_t[:], Alu.subtract)
                # z = KSC*uw + d in f32 (bf16 would destroy d at |z|~600)
                nc.vector.scalar_tensor_tensor(
                    z_t[:], uw_t[:], KSC, d_t[:], Alu.mult, Alu.add
                )

                for t in range(n_dve):
                    col = slice(it * n_dve + t, it * n_dve + t + 1)
                    nc.vector.scalar_tensor_tensor(
                        scr_t[:], uw_t[:], float(127 + t), d_t[:],
                        Alu.is_gt, Alu.mult, accum_out=Ldve[:, col],
                    )
                for j in range(N_ACT):
                    t = n_dve + j
                    col = slice(it * N_ACT + j, it * N_ACT + j + 1)
                    # A_t = sum relu(z - KSC*(127.5+t)) = KSC*B_t + L_t
                    # B_t = sum relu(uw - (127.5+t))
                    nc.scalar.activation(
                        ascr_t[:], z_t[:], Act.Relu,
                        bias=biasA[t][:], accum_out=LA[:, col],
                    )
                    nc.scalar.activation(
                        ascr_t[:], uw_t[:], Act.Relu,
                        bias=biasB[t][:], accum_out=LB[:, col],
                    )

            nc.sync.dma_start(outL[:, :], Ldve[:])
            nc.sync.dma_start(outA[:, :], LA[:])
            nc.sync.dma_start(outB[:, :], LB[:])

    # Bacc lowering: splits multi-wait sync conditions into EventSemaphore
    # instructions (the HW encodes one wait per instruction) and the rest
    # of the pre-walrus pipeline.
    nc.compile()
    return nc, n_tiles


def _get_program():
    if "prog" not in _CACHE:
        _CACHE["prog"] = _build_program()
    return _CACHE["prog"]


def _combine(partL_list, partA_list, partB_list, total):
    n_dve = N_BINS - N_ACT
    L = np.zeros(N_BINS, dtype=np.float64)
    for pl, pa, pb in zip(partL_list, partA_list, partB_list):
        pl = np.asarray(pl).astype(np.float64)
        L[:n_dve] += pl.reshape(pl.shape[0], -1, n_dve).sum(axis=(0, 1))
        pa = np.asarray(pa).astype(np.float64)
        pb = np.asarray(pb).astype(np.float64)
        A = pa.reshape(pa.shape[0], -1, N_ACT).sum(axis=(0, 1))
        B = pb.reshape(pb.shape[0], -1, N_ACT).sum(axis=(0, 1))
        L[n_dve:] += A - KSC * B
    delta = L.copy()
    delta[:-1] -= L[1:]
    return np.float32(np.abs(delta).sum() / total)


def kernel(confidences, predictions, targets, mask):
    global LAST_EXEC_TIME_NS, LAST_RESULTS
    nc, n_tiles = _get_program()

    conf = np.ascontiguousarray(np.asarray(confidences, dtype=np.float32))
    pred = np.asarray(predictions, dtype=np.int32)
    targ = np.asarray(targets, dtype=np.int32)
    msk = np.ascontiguousarray(np.asarray(mask, dtype=np.int32))
    assert conf.shape == (FULL_ROWS, COLS)

    pt = np.ascontiguousarray(np.stack([pred, targ], axis=1))

    in_maps = []
    for i in range(N_CORES):
        sl = slice(i * ROWS, (i + 1) * ROWS)
        in_maps.append({"confidences": conf[sl], "pt": pt[sl], "wm": msk[sl]})

    trace = bool(int(os.environ.get("ECE_TRACE", "0")))
    res = run_bass_kernel_spmd(nc, in_maps, list(range(N_CORES)), trace=trace)
    LAST_EXEC_TIME_NS = res.exec_time_ns
    LAST_RESULTS = res

    total = float(msk.sum(dtype=np.int64))
    return _combine(
        [res.results[i]["partL"] for i in range(N_CORES)],
        [res.results[i]["partA"] for i in range(N_CORES)],
        [res.results[i]["partB"] for i in range(N_CORES)],
        total,
    )
